# revision 6
# baseline (speedup 1.0000x reference)
"""GCN diag-encoder (2-layer SpMM) on 8 Trainium2 NeuronCores.

Strategy: the sparse adjacency (640K edges over 10K nodes, ~0.64% dense) is
materialized as a dense A^T matrix on the host and the per-layer
  out[dst] = sum_e vals[e] * x[src[e]]        (segment-sum SpMM)
becomes a dense matmul  out1_t = x^T-tiles contracted with A^T-tiles on the
TensorEngine:
  matmul(out=psum[feat, dst], lhsT=x_tile[src,feat], rhs=AT_tile[src,dst])
Each core owns a 1250-column (dst) slice of A^T (padded to 1280) and
accumulates its whole [128 feat x 1280 dst] output in PSUM while streaming
A^T k-tiles from HBM in bf16.  Between the two layers: tanh on the scalar
engine, diag-W scale, PE-transpose to node-major, AllGather across the 8
cores, then layer 2 with a row-permuted copy of A^T matched to the gathered
layout.  W (diag weights) are folded: W0 into x on the host, W1 applied at
the layer-1 eviction.
"""

import numpy as np
import ml_dtypes

N = 10000          # nodes
D = 128            # feature dim
NCORES = 8
S = 1250           # dst nodes per core
SP = 1280          # padded dst per core (10 tiles of 128)
KT = 80            # contraction k-tiles (padded src rows = 10240)
NPAD = KT * 128    # 10240
G = 10             # k-tile groups per layer (DMA batches)
KPG = 8            # k-tiles per group
BF16 = ml_dtypes.bfloat16

_PROG_CACHE = {}


def _build_program(groups=G):
    import concourse.bacc as bacc
    import concourse.mybir as mybir
    from concourse import tile

    f32 = mybir.dt.float32
    bf16 = mybir.dt.bfloat16

    nc = bacc.Bacc(
        "TRN2",
        target_bir_lowering=False,
        debug=False,
        enable_asserts=False,
        num_devices=NCORES,
    )

    a1 = nc.dram_tensor("a1", [G, 128, KPG * SP], bf16, kind="ExternalInput").ap()
    a2 = nc.dram_tensor("a2", [G, 128, KPG * SP], bf16, kind="ExternalInput").ap()
    x0 = nc.dram_tensor("x0", [128, NPAD], bf16, kind="ExternalInput").ap()
    w1 = nc.dram_tensor("w1", [128, 1], f32, kind="ExternalInput").ap()
    ident = nc.dram_tensor("ident", [128, 128], f32, kind="ExternalInput").ap()
    out = nc.dram_tensor("out", [128, SP], f32, kind="ExternalOutput").ap()

    with tile.TileContext(nc) as tc:
        with (
            tc.tile_pool(name="xp", bufs=1) as xp,
            tc.tile_pool(name="ab", bufs=3) as apool,
            tc.tile_pool(name="ev", bufs=1) as ev,
            tc.tile_pool(name="ps", bufs=1, space="PSUM") as ps,
            tc.tile_pool(name="pt", bufs=2, space="PSUM") as pt,
            tc.tile_pool(name="dr", bufs=1, space="DRAM") as dr,
        ):
            x0s = xp.tile([128, NPAD], bf16, tag="x0s")
            x1s = xp.tile([128, NPAD], bf16, tag="x1s")
            w1c = xp.tile([128, 1], f32, tag="w1c")
            idn = xp.tile([128, 128], f32, tag="idn")
            nc.scalar.dma_start(x0s[:], x0)
            nc.scalar.dma_start(w1c[:], w1)
            nc.scalar.dma_start(idn[:], ident)

            agin = dr.tile([128, SP], bf16)
            agout = dr.tile([NCORES * 128, SP], bf16, addr_space="Shared")

            def do_layer(a_dram, xs, psum):
                for g in range(groups):
                    ab = apool.tile([128, KPG * SP], bf16, tag="ab")
                    nc.sync.dma_start(ab[:], a_dram[g])
                    for kk in range(KPG):
                        k = g * KPG + kk
                        lhsT = xs[:, k * 128:(k + 1) * 128]
                        for c0, cn in ((0, 512), (512, 512), (1024, 256)):
                            nc.tensor.matmul(
                                psum[:, c0:c0 + cn],
                                lhsT,
                                ab[:, kk * SP + c0: kk * SP + c0 + cn],
                                start=(k == 0),
                                stop=(k == groups * KPG - 1),
                            )

            # ---- layer 1 ----
            psum1 = ps.tile([128, SP], f32, tag="acc1")
            do_layer(a1, x0s, psum1)

            # evict: x1 = tanh(psum1) * W1  (feat on partitions -> W1 is a
            # per-partition scalar), then transpose each 128x128 tile to
            # node-major for the AllGather.
            x1f = ev.tile([128, SP], f32, tag="x1f")
            nc.scalar.activation(
                x1f[:], psum1[:], mybir.ActivationFunctionType.Tanh
            )
            nc.vector.tensor_scalar_mul(x1f[:], x1f[:], w1c[:])
            agin_sb = ev.tile([128, SP], bf16, tag="agin")
            for t in range(10):
                tp = pt.tile([128, 128], f32, tag="tp")
                nc.tensor.transpose(tp[:], x1f[:, t * 128:(t + 1) * 128], idn[:])
                nc.vector.tensor_copy(agin_sb[:, t * 128:(t + 1) * 128], tp[:])
            nc.scalar.dma_start(agin[:], agin_sb[:])

            nc.gpsimd.collective_compute(
                "AllGather",
                mybir.AluOpType.bypass,
                replica_groups=[list(range(NCORES))],
                ins=[agin.opt()],
                outs=[agout.opt()],
            )
            # agout rows r*128..(r+1)*128 = rank r's node-major shard; lay
            # them side-by-side in the free dim to form layer-2 lhsT tiles.
            nc.sync.dma_start(
                x1s[:].rearrange("p (r j) -> p r j", r=NCORES),
                agout[:].rearrange("(r p) j -> p r j", p=128),
            )

            # ---- layer 2 ----
            psum2 = ps.tile([128, SP], f32, tag="acc2")
            do_layer(a2, x1s, psum2)
            ob = ev.tile([128, SP], f32, tag="ob")
            nc.vector.tensor_copy(ob[:], psum2[:])
            nc.sync.dma_start(out, ob[:])

    nc.compile()
    return nc


def get_program(groups=G):
    key = ("nc", groups)
    if key not in _PROG_CACHE:
        _PROG_CACHE[key] = _build_program(groups)
    return _PROG_CACHE[key]


def build_in_maps(x, src, dst, vals, W):
    """Host-side prep: dense A^T shards (bf16) + arranged x0."""
    import scipy.sparse as sp

    x = np.asarray(x, np.float32)
    src = np.asarray(src, np.int64)
    dst = np.asarray(dst, np.int64)
    vals = np.asarray(vals, np.float32)
    W = np.asarray(W, np.float32)

    # A[dst, src] = sum of vals  ->  we build AT[src, dst]
    AT = sp.coo_matrix((vals, (src, dst)), shape=(N, N)).toarray()

    xw = x * W[0][None, :]
    x0h = np.zeros((NPAD, D), np.float32)
    x0h[:N] = xw
    # [p, (k f)] layout: col-block k holds feats of node k*128+p
    x0h = np.ascontiguousarray(
        x0h.reshape(KT, 128, D).transpose(1, 0, 2).reshape(128, KT * D)
    ).astype(BF16)

    # layer-2 contraction-row permutation: row i <- global node of
    # (rank r = i//1280, tile t = (i%1280)//128, p = i%128)
    i2 = np.arange(NPAD)
    r2 = i2 // SP
    loc = i2 % SP
    node2 = r2 * S + loc
    valid2 = loc < S
    node2c = np.where(valid2, node2, 0)

    w1col = np.ascontiguousarray(W[1].reshape(128, 1)).astype(np.float32)
    ident = np.eye(128, dtype=np.float32)

    def arrange(a_pad16):
        # [NPAD, SP] -> [G, 128, KPG*SP] with [g, p, kk*SP+j] = row g*1024+kk*128+p
        return np.ascontiguousarray(
            a_pad16.reshape(G, KPG, 128, SP).transpose(0, 2, 1, 3).reshape(
                G, 128, KPG * SP
            )
        )

    in_maps = []
    for c in range(NCORES):
        ATc = np.zeros((NPAD, SP), np.float32)
        ATc[:N, :S] = AT[:, c * S:(c + 1) * S]
        ATc16 = ATc.astype(BF16)
        AT2 = ATc16[node2c]
        AT2[~valid2] = 0
        in_maps.append(
            {
                "a1": arrange(ATc16),
                "a2": arrange(AT2),
                "x0": x0h,
                "w1": w1col,
                "ident": ident,
            }
        )
    return in_maps


def assemble_output(results):
    outs = []
    for c in range(NCORES):
        ot = np.asarray(results[c]["out"], np.float32)  # [128, SP] feat-major
        outs.append(ot[:, :S].T)
    return np.ascontiguousarray(np.concatenate(outs, axis=0))


def kernel(x, src, dst, vals, W):
    from concourse import bass_utils

    nc = get_program()
    in_maps = build_in_maps(x, src, dst, vals, W)
    res = bass_utils.run_bass_kernel_spmd(
        nc, in_maps, core_ids=list(range(NCORES))
    )
    return assemble_output(res.results)


# revision 29
# speedup vs baseline: 1.0013x; 1.0013x over previous
"""GCN diag-encoder (2-layer SpMM) on 8 Trainium2 NeuronCores.

Strategy: the sparse adjacency (640K edges over 10K nodes, ~0.64% dense) is
materialized as a dense A^T matrix on the host; each per-layer
  out[dst] = sum_e vals[e] * x[src[e]]        (segment-sum SpMM)
becomes dense TensorEngine matmuls
  matmul(out=psum[feat, dst], lhsT=x_tile[src,feat], rhs=AT_tile[src,dst]).
Each core owns a 1250-wide dst slice of A^T (padded to 1280) and accumulates
its whole [128 feat x 1280 dst] output in PSUM while streaming A^T k-tiles
from HBM (uint8-quantized, cast to bf16 in the DMA).  Src nodes use a
padded rank-block ordering (rank r owns slots r*1280..r*1280+1279, 30 pad
slots per rank) so layer 2's gathered activations line up with the SAME A
arrangement as layer 1 — which also lets the first R k-tile groups of A stay
resident in SBUF for layer 2.  Between layers: tanh (+dequant scale) on the
scalar engine, diag-W1 scale, PE-transpose to node-major, AllGather across
the 8 cores.  W0 is folded into x on the host.
"""

import numpy as np
import ml_dtypes

N = 10000          # nodes
D = 128            # feature dim
NCORES = 8
S = 1250           # dst nodes per core
SP = 1280          # padded dst per core (10 tiles of 128)
KT = 80            # contraction k-tiles (padded src rows = 10240)
NPAD = KT * 128    # 10240
G = 10             # k-tile groups per layer (DMA batches)
KPG = 8            # k-tiles per group
RES = 4            # A groups kept resident in SBUF for layer 2
BF16 = ml_dtypes.bfloat16

_PROG_CACHE = {}


def _build_program(groups=G, nocc=False, skip=(), u8=True, res=RES, abufs=3):
    import concourse.bacc as bacc
    import concourse.mybir as mybir
    from concourse import tile

    f32 = mybir.dt.float32
    bf16 = mybir.dt.float16
    adt = mybir.dt.uint8 if u8 else bf16
    res = min(res, groups)

    nc = bacc.Bacc(
        "TRN2",
        target_bir_lowering=False,
        debug=False,
        enable_asserts=False,
        num_devices=1 if nocc else NCORES,
    )

    a = nc.dram_tensor("a", [G, 128, KPG * SP], adt, kind="ExternalInput").ap()
    x0 = nc.dram_tensor("x0", [128, NPAD], bf16, kind="ExternalInput").ap()
    w1 = nc.dram_tensor("w1", [128, 1], f32, kind="ExternalInput").ap()
    cs = nc.dram_tensor("cs", [128, SP], f32, kind="ExternalInput").ap()
    ident = nc.dram_tensor("ident", [128, 128], f32, kind="ExternalInput").ap()
    out = nc.dram_tensor("out", [128, SP], f32, kind="ExternalOutput").ap()

    with tile.TileContext(nc) as tc:
        with (
            tc.tile_pool(name="xp", bufs=1) as xp,
            tc.tile_pool(name="ab", bufs=abufs) as apool,
            tc.tile_pool(name="res", bufs=1) as rpool,
            tc.tile_pool(name="ev", bufs=1) as ev,
            tc.tile_pool(name="ps", bufs=1, space="PSUM") as ps,
            tc.tile_pool(name="pt", bufs=2, space="PSUM") as pt,
            tc.tile_pool(name="dr", bufs=1, space="DRAM") as dr,
        ):
            x0s = xp.tile([128, NPAD], bf16, tag="x0s")
            x1s = xp.tile([128, NPAD], bf16, tag="x1s")
            w1c = xp.tile([128, 1], f32, tag="w1c")
            cst = xp.tile([128, SP], f32, tag="cst")
            idn = xp.tile([128, 128], f32, tag="idn")
            nc.scalar.dma_start(x0s[:], x0)
            nc.scalar.dma_start(w1c[:], w1)
            nc.scalar.dma_start(cst[:], cs)
            nc.scalar.dma_start(idn[:], ident)

            agin = dr.tile([128, SP], bf16)
            agout = dr.tile([NCORES * 128, SP], bf16, addr_space="Shared")

            res_tiles = {}

            def fetch_group(g):
                """DMA group g of A into an SBUF tile (bf16, cast if u8)."""
                if g < res:
                    ab = rpool.tile([128, KPG * SP], bf16, tag=f"res{g}")
                    res_tiles[g] = ab
                else:
                    ab = apool.tile([128, KPG * SP], bf16, tag="ab")
                if "adma" in skip:
                    nc.gpsimd.dma_start(ab[:, 0:8], a[g][:, 0:8])
                elif u8:
                    nc.gpsimd.dma_start(ab[:], a[g])
                else:
                    nc.sync.dma_start(ab[:], a[g])
                return ab

            def mm_group(g, ab, xs, psum, kidx):
                for kk in range(KPG):
                    k = g * KPG + kk
                    lhsT = xs[:, k * 128:(k + 1) * 128]
                    for c0, cn in ((0, 512), (512, 512), (1024, 256)):
                        nc.tensor.matmul(
                            psum[:, c0:c0 + cn],
                            lhsT,
                            ab[:, kk * SP + c0: kk * SP + c0 + cn],
                            start=(kidx == 0),
                            stop=(kidx == groups * KPG - 1),
                        )
                    kidx += 1
                return kidx

            # ---- layer 1: stream every group (first `res` land in
            # resident tiles and stay for layer 2) ----
            psum1 = ps.tile([128, SP], f32, tag="acc1")
            kidx = 0
            for g in range(groups):
                ab = fetch_group(g)
                kidx = mm_group(g, ab, x0s, psum1, kidx)

            # evict: x1 = tanh(ascale*psum1) * W1 (feat on partitions ->
            # per-partition scalars), then PE-transpose each 128x128 tile
            # to node-major for the AllGather.
            x1f = ev.tile([128, SP], f32, tag="x1f")
            nc.vector.tensor_mul(x1f[:], psum1[:], cst[:])
            nc.scalar.activation(
                x1f[:], x1f[:], mybir.ActivationFunctionType.Tanh,
            )
            nc.vector.tensor_scalar_mul(x1f[:], x1f[:], w1c[:])
            agin_sb = ev.tile([128, SP], bf16, tag="agin")
            for t in range(10):
                tp = pt.tile([128, 128], f32, tag="tp")
                nc.tensor.transpose(tp[:], x1f[:, t * 128:(t + 1) * 128], idn[:])
                nc.vector.tensor_copy(agin_sb[:, t * 128:(t + 1) * 128], tp[:])
            nc.scalar.dma_start(agin[:], agin_sb[:])

            if nocc:
                nc.scalar.dma_start(agout[0:128, :], agin[:])
            else:
                nc.gpsimd.collective_compute(
                    "AllGather",
                    mybir.AluOpType.bypass,
                    replica_groups=[list(range(NCORES))],
                    ins=[agin.opt()],
                    outs=[agout.opt()],
                )
            # agout rows r*128..(r+1)*128 = rank r's node-major shard; side
            # by side in the free dim they are exactly layer-2's lhsT tiles
            # in the same padded rank-block order A uses.
            nc.sync.dma_start(
                x1s[:].rearrange("p (r j) -> p r j", r=NCORES),
                agout[:].rearrange("(r p) j -> p r j", p=128),
            )

            # ---- layer 2: resident groups first (PE can start while the
            # re-streamed groups arrive) ----
            psum2 = ps.tile([128, SP], f32, tag="acc2")
            kidx = 0
            for g in range(groups):
                ab = res_tiles[g] if g < res else fetch_group(g)
                kidx = mm_group(g, ab, x1s, psum2, kidx)

            ob = ev.tile([128, SP], f32, tag="ob")
            nc.vector.tensor_mul(ob[:], psum2[:], cst[:])
            nc.sync.dma_start(out, ob[:])

    nc.compile()
    return nc


def get_program(groups=G, nocc=False, skip=(), u8=True, res=RES, abufs=3):
    key = ("nc", groups, nocc, tuple(skip), u8, res, abufs)
    if key not in _PROG_CACHE:
        _PROG_CACHE[key] = _build_program(groups, nocc, skip, u8, res, abufs)
    return _PROG_CACHE[key]


def _node_perm():
    """Padded rank-block src ordering: slot i <-> (rank r = i//1280,
    local q = i%1280); global node r*1250+q for q<1250, else pad."""
    i2 = np.arange(NPAD)
    r2 = i2 // SP
    loc = i2 % SP
    node = r2 * S + loc
    valid = loc < S
    return np.where(valid, node, 0), valid


def build_in_maps(x, src, dst, vals, W, u8=True):
    """Host-side prep: dense A^T shard (u8-quantized or bf16) + arranged x0."""
    import scipy.sparse as sp

    x = np.asarray(x, np.float32)
    src = np.asarray(src, np.int64)
    dst = np.asarray(dst, np.int64)
    vals = np.asarray(vals, np.float32)
    W = np.asarray(W, np.float32)

    # A[dst, src] = sum of vals  ->  we build AT[src, dst]
    AT = sp.coo_matrix((vals, (src, dst)), shape=(N, N)).toarray()

    node2, valid2 = _node_perm()

    xw = x * W[0][None, :]
    x0p = np.zeros((NPAD, D), np.float32)
    x0p[valid2] = xw[node2[valid2]]
    x0h = np.ascontiguousarray(
        x0p.reshape(KT, 128, D).transpose(1, 0, 2).reshape(128, KT * D)
    ).astype(np.float16)

    w1col = np.ascontiguousarray(W[1].reshape(128, 1)).astype(np.float32)
    ident = np.eye(128, dtype=np.float32)

    def arrange(a_pad):
        # [NPAD, SP] -> [G, 128, KPG*SP] with [g, p, kk*SP+j] = row g*1024+kk*128+p
        return np.ascontiguousarray(
            a_pad.reshape(G, KPG, 128, SP).transpose(0, 2, 1, 3).reshape(
                G, 128, KPG * SP
            )
        )

    in_maps = []
    for c in range(NCORES):
        ATc = AT[:, c * S:(c + 1) * S]  # [N, S] float32
        colmax = np.maximum(ATc.max(axis=0), 1e-9)
        step = colmax / 255.0
        if u8:
            Aq = np.clip(np.rint(ATc * (1.0 / step)[None, :]), 0, 255).astype(
                np.uint8
            )
        else:
            Aq = (ATc * (1.0 / step)[None, :]).astype(np.float16)
        Ap = np.zeros((NPAD, SP), Aq.dtype)
        Ap[valid2, :S] = Aq[node2[valid2]]
        cs_tile = np.zeros((128, SP), np.float32)
        cs_tile[:, :S] = step[None, :]
        in_maps.append(
            {
                "a": arrange(Ap),
                "x0": x0h,
                "w1": w1col,
                "cs": cs_tile,
                "ident": ident,
            }
        )
    return in_maps


def assemble_output(results):
    outs = []
    for c in range(NCORES):
        ot = np.asarray(results[c]["out"], np.float32)  # [128, SP] feat-major
        outs.append(ot[:, :S].T)
    return np.ascontiguousarray(np.concatenate(outs, axis=0))


def kernel(x, src, dst, vals, W):
    from concourse import bass_utils

    nc = get_program()
    in_maps = build_in_maps(x, src, dst, vals, W)
    # The axon terminal can wedge when a different program was loaded
    # earlier in its lifetime; a retry lands on the restarted terminal.
    last_err = None
    for _attempt in range(3):
        try:
            res = bass_utils.run_bass_kernel_spmd(
                nc, in_maps, core_ids=list(range(NCORES))
            )
            return assemble_output(res.results)
        except Exception as e:  # noqa: BLE001
            last_err = e
            import time as _time

            _time.sleep(10.0)
    raise last_err


# revision 44
# speedup vs baseline: 580.7974x; 580.0557x over previous
"""GCN diag-encoder (2-layer SpMM) on 8 Trainium2 NeuronCores.

Strategy: the sparse adjacency (640K edges over 10K nodes, ~0.64% dense) is
materialized as a dense A^T matrix on the host; each per-layer
  out[dst] = sum_e vals[e] * x[src[e]]        (segment-sum SpMM)
becomes dense TensorEngine matmuls.  Each core owns a 1250-wide dst slice of
A^T (padded to 1280, uint8-quantized per dst column) and streams A^T k-tiles
from HBM with an inline u8->f16 cast in the DMA, in variable-size k-tile
groups (small first/last groups shorten the pipeline ramp and tail).

Layer 1 runs A-stationary — matmul(out=psum[dst,feat], lhsT=AT_tile[src,dst],
rhs=x_tile[src,feat]) — so the layer-1 output is already node-major: the
eviction is a fused tanh+dequant-scale pass on the scalar engine (scale is
per dst node = per partition) straight into the AllGather bounce.  PSUM
accumulation groups are per 2KiB bank while layer 1 writes four 512B ranges
per bank, so each bank is seeded by one full-width start=True zero matmul.
Layer 2 (PE-bound) runs X-stationary — matmul(out=psum[feat,dst],
lhsT=x1_tile[src,feat], rhs=AT_tile[src,dst]); its dequant scale (per dst =
per free element) and the final transpose are applied on the host.

Src nodes use a padded rank-block ordering (rank r owns slots
r*1280..r*1280+1279) so layer 2's AllGathered activations line up with the
SAME A arrangement layer 1 uses — the first RESG k-tile groups of A stay
resident in SBUF for layer 2, and layer 2 interleaves resident/streamed
groups so PE starts on the earliest-arriving x1 chunks while the remaining
A-stream DMAs land.  W0 is folded into x on the host; W1 is skipped on
device when it is all-ones (torch init), else applied via a broadcast
multiply.
"""

import numpy as np
import ml_dtypes

N = 10000          # nodes
D = 128            # feature dim
NCORES = 8
S = 1250           # dst nodes per core
SP = 1280          # padded dst per core (10 tiles of 128)
KT = 80            # contraction k-tiles (padded src rows = 10240)
NPAD = KT * 128    # 10240
GSIZES = (8, 8, 8, 8, 8, 8, 8, 8, 8, 8)   # k-tiles per group
RESG = 6           # leading groups kept resident in SBUF for layer 2
BF16 = ml_dtypes.bfloat16

_PROG_CACHE = {}


def _groups():
    out = []
    k0 = 0
    for sz in GSIZES:
        out.append((k0, k0 + sz))
        k0 += sz
    assert k0 == KT
    return out


def _build_program(nocc=False, skip=(), u8=True, resg=RESG, abufs=2,
                   w1_ones=True, l2order="streamfirst", gsizes=GSIZES):
    import concourse.bacc as bacc
    import concourse.mybir as mybir
    from concourse import tile

    f32 = mybir.dt.float32
    f16 = mybir.dt.float16
    adt = mybir.dt.uint8 if u8 else f16
    grps = []
    _k0 = 0
    for _sz in gsizes:
        grps.append((_k0, _k0 + _sz))
        _k0 += _sz
    assert _k0 == KT
    maxg = max(k1 - k0 for k0, k1 in grps)

    nc = bacc.Bacc(
        "TRN2",
        target_bir_lowering=False,
        debug=False,
        enable_asserts=False,
        num_devices=1 if nocc else NCORES,
    )

    a = nc.dram_tensor("a", [KT, 128, SP], adt, kind="ExternalInput").ap()
    x0 = nc.dram_tensor("x0", [128, NPAD], f16, kind="ExternalInput").ap()
    # per-dst-node dequant scales, [slot p, tile t] layout
    csc = nc.dram_tensor("csc", [128, 10], f32, kind="ExternalInput").ap()
    # broadcast W1 row (only read when not w1_ones)
    w1b = nc.dram_tensor("w1b", [128, 128], f16, kind="ExternalInput").ap()
    out = nc.dram_tensor("out", [128, SP], f32, kind="ExternalOutput").ap()

    with tile.TileContext(nc) as tc:
        with (
            tc.tile_pool(name="xp", bufs=1) as xp,
            tc.tile_pool(name="ab", bufs=abufs) as apool,
            tc.tile_pool(name="res", bufs=1) as rpool,
            tc.tile_pool(name="ev", bufs=1) as ev,
            tc.tile_pool(name="ps", bufs=1, space="PSUM") as ps,
            tc.tile_pool(name="dr", bufs=1, space="DRAM") as dr,
        ):
            # x0 is dead once layer 1 finishes; share one slot for both
            x0s = xp.tile([128, NPAD], f16, tag="xs")
            x1s = xp.tile([128, NPAD], f16, tag="xs")
            cscs = xp.tile([128, 10], f32, tag="cscs")
            w1s = xp.tile([128, 128], f16, tag="w1s")
            zl = xp.tile([128, 512], f16, tag="zl")
            warm = xp.tile([128, 1], f32, tag="warm")
            nc.scalar.dma_start(cscs[:], csc)
            if not w1_ones:
                nc.scalar.dma_start(w1s[:], w1b)
            nc.vector.memset(zl[:], 0.0)
            # pre-load the ACT tanh table so the layer-1 eviction doesn't
            # pay the table load on the critical path
            nc.scalar.activation(
                warm[:], zl[:, 0:1], mybir.ActivationFunctionType.Tanh
            )

            agin = dr.tile([128, SP], f16)
            agout = dr.tile([NCORES * 128, SP], f16, addr_space="Shared")

            res_tiles = {}

            def fetch_group(gi):
                """DMA group gi of A into an SBUF tile (f16, cast if u8)."""
                k0, k1 = grps[gi]
                nk = k1 - k0
                if gi < resg:
                    ab = rpool.tile([128, nk * SP], f16, tag=f"res{gi}")
                    res_tiles[gi] = ab
                else:
                    ab = apool.tile([128, maxg * SP], f16, tag="ab")
                src = a[k0:k1].rearrange("k p j -> p k j")
                dst = ab[:, 0:nk * SP].rearrange("p (k j) -> p k j", k=nk)
                if "adma" in skip:
                    nc.gpsimd.dma_start(ab[:, 0:8], a[k0][:, 0:8])
                elif u8:
                    nc.gpsimd.dma_start(dst, src)
                else:
                    nc.sync.dma_start(dst, src)
                return ab

            # ---- layer 1 (A-stationary; psum is [dst slot, feat]) ----
            psum1 = ps.tile([128, SP], f32, tag="acc1")
            for c0, cn in ((0, 512), (512, 512), (1024, 256)):
                nc.tensor.matmul(
                    psum1[:, c0:c0 + cn], zl[:, 0:128], zl[:, 0:cn],
                    start=True, stop=False,
                )
            for gi, (k0, k1) in enumerate(grps):
                nc.scalar.dma_start(
                    x0s[:, k0 * 128:k1 * 128], x0[:, k0 * 128:k1 * 128]
                )
                ab = fetch_group(gi)
                for k in range(k0, k1):
                    kk = k - k0
                    rhs = x0s[:, k * 128:(k + 1) * 128]
                    for t in range(10):
                        nc.tensor.matmul(
                            psum1[:, t * 128:(t + 1) * 128],
                            ab[:, kk * SP + t * 128: kk * SP + (t + 1) * 128],
                            rhs,
                            start=False,
                            stop=(k == KT - 1 and t in (3, 7, 9)),
                        )

            # evict layer 1: x1 = tanh(cs_dst * psum1) [* W1] on ACT, chunked
            # agin DMA so the AllGather input lands as soon as possible.
            agin_sb = ev.tile([128, SP], f16, tag="agin")
            for t in range(10):
                c0, c1 = t * 128, (t + 1) * 128
                nc.scalar.activation(
                    agin_sb[:, c0:c1], psum1[:, c0:c1],
                    mybir.ActivationFunctionType.Tanh,
                    scale=cscs[:, t:t + 1],
                )
                if not w1_ones:
                    nc.vector.tensor_mul(
                        agin_sb[:, c0:c1], agin_sb[:, c0:c1], w1s[:]
                    )
                nc.scalar.dma_start(agin[:, c0:c1], agin_sb[:, c0:c1])

            if nocc:
                nc.scalar.dma_start(agout[0:128, :], agin[:])
            else:
                nc.gpsimd.collective_compute(
                    "AllGather",
                    mybir.AluOpType.bypass,
                    replica_groups=[list(range(NCORES))],
                    ins=[agin.opt()],
                    outs=[agout.opt()],
                )
            # agout rank blocks laid side by side in the free dim are exactly
            # layer-2's lhsT tiles in the same padded rank-block order A uses.
            for r in range(NCORES):
                nc.sync.dma_start(
                    x1s[:, r * SP:(r + 1) * SP],
                    agout[r * 128:(r + 1) * 128, :],
                )

            # ---- layer 2 (X-stationary; psum is [feat, dst]) ----
            # Interleave: residents first (rank-0 x1 chunk arrives first),
            # streamed groups spread out so their DMAs pipeline through the
            # abufs slots while PE chews residents.
            psum2 = ps.tile([128, SP], f32, tag="acc2")
            residents = list(range(resg))
            streams = list(range(resg, len(grps)))
            if l2order == "streamfirst":
                order = streams[:abufs] + residents + streams[abufs:]
            else:
                order = []
                ri, si = 0, 0
                pattern = [0, 0, 1, 0, 1, 0, 1, 0, 1, 0, 1, 0]  # 1 = stream
                for p in pattern[:len(grps)]:
                    if p and si < len(streams):
                        order.append(streams[si]); si += 1
                    elif ri < len(residents):
                        order.append(residents[ri]); ri += 1
                order += residents[ri:] + streams[si:]
                if order[-1] in streams:
                    for i in range(len(order) - 2, -1, -1):
                        if order[i] in residents:
                            order.append(order.pop(i))
                            break

            first = True
            for oi, gi in enumerate(order):
                k0, k1 = grps[gi]
                ab = res_tiles[gi] if gi < resg else fetch_group(gi)
                last_grp = oi == len(order) - 1
                for k in range(k0, k1):
                    kk = k - k0
                    lhsT = x1s[:, k * 128:(k + 1) * 128]
                    for c0, cn in ((0, 512), (512, 512), (1024, 256)):
                        nc.tensor.matmul(
                            psum2[:, c0:c0 + cn],
                            lhsT,
                            ab[:, kk * SP + c0: kk * SP + c0 + cn],
                            start=first,
                            stop=(last_grp and k == k1 - 1),
                        )
                    first = False

            # evict layer 2 raw; host applies per-dst dequant scale + final
            # transpose.  Chunked so copy and DMA pipeline.
            ob = ev.tile([128, SP], f32, tag="ob")
            for c0, cn in ((0, 512), (512, 512), (1024, 256)):
                nc.vector.tensor_copy(ob[:, c0:c0 + cn], psum2[:, c0:c0 + cn])
                nc.sync.dma_start(out[:, c0:c0 + cn], ob[:, c0:c0 + cn])

    nc.compile()
    return nc


def get_program(nocc=False, skip=(), u8=True, resg=RESG, abufs=2,
                w1_ones=True, l2order="streamfirst", gsizes=GSIZES):
    key = ("nc", nocc, tuple(skip), u8, resg, abufs, w1_ones, l2order,
           tuple(gsizes))
    if key not in _PROG_CACHE:
        _PROG_CACHE[key] = _build_program(nocc, skip, u8, resg, abufs,
                                          w1_ones, l2order, gsizes)
    return _PROG_CACHE[key]


def _node_perm():
    """Padded rank-block src ordering: slot i <-> (rank r = i//1280,
    local q = i%1280); global node r*1250+q for q<1250, else pad."""
    i2 = np.arange(NPAD)
    r2 = i2 // SP
    loc = i2 % SP
    node = r2 * S + loc
    valid = loc < S
    return np.where(valid, node, 0), valid


def build_in_maps(x, src, dst, vals, W, u8=True):
    """Host-side prep: dense A^T shard (u8 per-column quantized) + x0."""
    import scipy.sparse as sp

    x = np.asarray(x, np.float32)
    src = np.asarray(src, np.int64)
    dst = np.asarray(dst, np.int64)
    vals = np.asarray(vals, np.float32)
    W = np.asarray(W, np.float32)

    # A[dst, src] = sum of vals  ->  we build AT[src, dst]
    AT = sp.coo_matrix((vals, (src, dst)), shape=(N, N)).toarray()

    node2, valid2 = _node_perm()

    xw = x * W[0][None, :]
    x0p = np.zeros((NPAD, D), np.float32)
    x0p[valid2] = xw[node2[valid2]]
    x0h = np.ascontiguousarray(
        x0p.reshape(KT, 128, D).transpose(1, 0, 2).reshape(128, KT * D)
    ).astype(np.float16)

    w1brow = np.ascontiguousarray(
        np.tile(W[1][None, :], (128, 1))
    ).astype(np.float16)

    in_maps = []
    steps = []
    for c in range(NCORES):
        ATc = AT[:, c * S:(c + 1) * S]  # [N, S] float32
        colmax = np.maximum(ATc.max(axis=0), 1e-9)
        step = colmax / 255.0
        if u8:
            Aq = np.clip(np.rint(ATc * (1.0 / step)[None, :]), 0, 255).astype(
                np.uint8
            )
        else:
            Aq = (ATc * (1.0 / step)[None, :]).astype(np.float16)
        Ap = np.zeros((NPAD, SP), Aq.dtype)
        Ap[valid2, :S] = Aq[node2[valid2]]
        step_pad = np.zeros(SP, np.float32)
        step_pad[:S] = step
        steps.append(step_pad)
        # csc[p, t] = dequant scale of dst slot t*128+p
        csc_tile = np.ascontiguousarray(step_pad.reshape(10, 128).T).astype(
            np.float32
        )
        in_maps.append(
            {
                "a": np.ascontiguousarray(Ap.reshape(KT, 128, SP)),
                "x0": x0h,
                "csc": csc_tile,
                "w1b": w1brow,
            }
        )
    return in_maps, steps


def assemble_output(results, steps):
    outs = []
    for c in range(NCORES):
        ot = np.asarray(results[c]["out"], np.float32)  # [128, SP] feat-major
        ot = ot * steps[c][None, :]  # per-dst dequant (layer-2)
        outs.append(ot[:, :S].T)
    return np.ascontiguousarray(np.concatenate(outs, axis=0))


def kernel(x, src, dst, vals, W):
    from concourse import bass_utils

    w1_ones = bool(np.all(np.asarray(W)[1] == 1.0))
    nc = get_program(w1_ones=w1_ones)
    in_maps, steps = build_in_maps(x, src, dst, vals, W)
    # The axon terminal can wedge when a different program was loaded
    # earlier in its lifetime; a retry lands on the restarted terminal.
    last_err = None
    for _attempt in range(3):
        try:
            res = bass_utils.run_bass_kernel_spmd(
                nc, in_maps, core_ids=list(range(NCORES))
            )
            return assemble_output(res.results, steps)
        except Exception as e:  # noqa: BLE001
            last_err = e
            import time as _time

            _time.sleep(10.0)
    raise last_err


# revision 46
# speedup vs baseline: 586.6241x; 1.0100x over previous
"""GCN diag-encoder (2-layer SpMM) on 8 Trainium2 NeuronCores.

Strategy: the sparse adjacency (640K edges over 10K nodes, ~0.64% dense) is
materialized as a dense A^T matrix on the host; each per-layer
  out[dst] = sum_e vals[e] * x[src[e]]        (segment-sum SpMM)
becomes dense TensorEngine matmuls.  Each core owns a 1250-wide dst slice of
A^T (padded to 1280, uint8-quantized per dst column) and streams A^T k-tiles
from HBM with an inline u8->f16 cast in the DMA, in variable-size k-tile
groups (small first/last groups shorten the pipeline ramp and tail).

Layer 1 runs A-stationary — matmul(out=psum[dst,feat], lhsT=AT_tile[src,dst],
rhs=x_tile[src,feat]) — so the layer-1 output is already node-major: the
eviction is a fused tanh+dequant-scale pass on the scalar engine (scale is
per dst node = per partition) straight into the AllGather bounce.  PSUM
accumulation groups are per 2KiB bank while layer 1 writes four 512B ranges
per bank, so each bank is seeded by one full-width start=True zero matmul.
Layer 2 (PE-bound) runs X-stationary — matmul(out=psum[feat,dst],
lhsT=x1_tile[src,feat], rhs=AT_tile[src,dst]); its dequant scale (per dst =
per free element) and the final transpose are applied on the host.

Src nodes use a padded rank-block ordering (rank r owns slots
r*1280..r*1280+1279) so layer 2's AllGathered activations line up with the
SAME A arrangement layer 1 uses — the first RESG k-tile groups of A stay
resident in SBUF for layer 2, and layer 2 interleaves resident/streamed
groups so PE starts on the earliest-arriving x1 chunks while the remaining
A-stream DMAs land.  W0 is folded into x on the host; W1 is skipped on
device when it is all-ones (torch init), else applied via a broadcast
multiply.
"""

import numpy as np
import ml_dtypes

N = 10000          # nodes
D = 128            # feature dim
NCORES = 8
S = 1250           # dst nodes per core
SP = 1280          # padded dst per core (10 tiles of 128)
KT = 80            # contraction k-tiles (padded src rows = 10240)
NPAD = KT * 128    # 10240
GSIZES = (8, 8, 8, 8, 8, 8, 8, 8, 8, 8)   # k-tiles per group
RESG = 6           # leading groups kept resident in SBUF for layer 2
BF16 = ml_dtypes.bfloat16

_PROG_CACHE = {}


def _groups():
    out = []
    k0 = 0
    for sz in GSIZES:
        out.append((k0, k0 + sz))
        k0 += sz
    assert k0 == KT
    return out


def _build_program(nocc=False, skip=(), u8=True, resg=RESG, abufs=2,
                   w1_ones=True, l2order="streamfirst", gsizes=GSIZES):
    import concourse.bacc as bacc
    import concourse.mybir as mybir
    from concourse import tile

    f32 = mybir.dt.float32
    f16 = mybir.dt.float16
    adt = mybir.dt.uint8 if u8 else f16
    grps = []
    _k0 = 0
    for _sz in gsizes:
        grps.append((_k0, _k0 + _sz))
        _k0 += _sz
    assert _k0 == KT
    maxg = max(k1 - k0 for k0, k1 in grps)

    nc = bacc.Bacc(
        "TRN2",
        target_bir_lowering=False,
        debug=False,
        enable_asserts=False,
        num_devices=1 if nocc else NCORES,
    )

    a = nc.dram_tensor("a", [KT, 128, SP], adt, kind="ExternalInput").ap()
    x0 = nc.dram_tensor("x0", [128, NPAD], f16, kind="ExternalInput").ap()
    # per-dst-node dequant scales, [slot p, tile t] layout
    csc = nc.dram_tensor("csc", [128, 10], f32, kind="ExternalInput").ap()
    # broadcast W1 row (only read when not w1_ones)
    w1b = nc.dram_tensor("w1b", [128, 128], f16, kind="ExternalInput").ap()
    out = nc.dram_tensor("out", [128, SP], f32, kind="ExternalOutput").ap()

    with tile.TileContext(nc) as tc:
        with (
            tc.tile_pool(name="xp", bufs=1) as xp,
            tc.tile_pool(name="ab", bufs=abufs) as apool,
            tc.tile_pool(name="res", bufs=1) as rpool,
            tc.tile_pool(name="ev", bufs=1) as ev,
            tc.tile_pool(name="ps", bufs=1, space="PSUM") as ps,
            tc.tile_pool(name="dr", bufs=1, space="DRAM") as dr,
        ):
            # x0 is dead once layer 1 finishes; share one slot for both
            x0s = xp.tile([128, NPAD], f16, tag="xs")
            x1s = xp.tile([128, NPAD], f16, tag="xs")
            cscs = xp.tile([128, 10], f32, tag="cscs")
            w1s = xp.tile([128, 128], f16, tag="w1s")
            zl = xp.tile([128, 512], f16, tag="zl")
            warm = xp.tile([128, 1], f32, tag="warm")
            nc.scalar.dma_start(cscs[:], csc)
            if not w1_ones:
                nc.scalar.dma_start(w1s[:], w1b)
            nc.vector.memset(zl[:], 0.0)
            # pre-load the ACT tanh table so the layer-1 eviction doesn't
            # pay the table load on the critical path
            nc.scalar.activation(
                warm[:], zl[:, 0:1], mybir.ActivationFunctionType.Tanh
            )

            agin = dr.tile([128, SP], f16)
            agout = dr.tile([NCORES * 128, SP], f16, addr_space="Shared")

            res_tiles = {}

            def fetch_group(gi, halves=1):
                """DMA group gi of A into an SBUF tile (f16, cast if u8)."""
                k0, k1 = grps[gi]
                nk = k1 - k0
                if gi < resg:
                    ab = rpool.tile([128, nk * SP], f16, tag=f"res{gi}")
                    res_tiles[gi] = ab
                else:
                    ab = apool.tile([128, maxg * SP], f16, tag="ab")
                if "adma" in skip:
                    nc.gpsimd.dma_start(ab[:, 0:8], a[k0][:, 0:8])
                    return ab
                bounds = [k0 + (nk * h) // halves for h in range(halves + 1)]
                for b0, b1 in zip(bounds, bounds[1:]):
                    if b0 == b1:
                        continue
                    src = a[b0:b1].rearrange("k p j -> p k j")
                    dst = ab[:, (b0 - k0) * SP:(b1 - k0) * SP].rearrange(
                        "p (k j) -> p k j", k=b1 - b0
                    )
                    if u8:
                        nc.gpsimd.dma_start(dst, src)
                    else:
                        nc.sync.dma_start(dst, src)
                return ab

            # ---- layer 1 (A-stationary; psum is [dst slot, feat]) ----
            psum1 = ps.tile([128, SP], f32, tag="acc1")
            for c0, cn in ((0, 512), (512, 512), (1024, 256)):
                nc.tensor.matmul(
                    psum1[:, c0:c0 + cn], zl[:, 0:128], zl[:, 0:cn],
                    start=True, stop=False,
                )
            for gi, (k0, k1) in enumerate(grps):
                nc.scalar.dma_start(
                    x0s[:, k0 * 128:k1 * 128], x0[:, k0 * 128:k1 * 128]
                )
                ab = fetch_group(gi, halves=2 if gi == 0 else 1)
                for k in range(k0, k1):
                    kk = k - k0
                    rhs = x0s[:, k * 128:(k + 1) * 128]
                    for t in range(10):
                        nc.tensor.matmul(
                            psum1[:, t * 128:(t + 1) * 128],
                            ab[:, kk * SP + t * 128: kk * SP + (t + 1) * 128],
                            rhs,
                            start=False,
                            stop=(k == KT - 1 and t in (3, 7, 9)),
                        )

            # evict layer 1: x1 = tanh(cs_dst * psum1) [* W1] on ACT, chunked
            # agin DMA so the AllGather input lands as soon as possible.
            agin_sb = ev.tile([128, SP], f16, tag="agin")
            for t in range(10):
                c0, c1 = t * 128, (t + 1) * 128
                nc.scalar.activation(
                    agin_sb[:, c0:c1], psum1[:, c0:c1],
                    mybir.ActivationFunctionType.Tanh,
                    scale=cscs[:, t:t + 1],
                )
                if not w1_ones:
                    nc.vector.tensor_mul(
                        agin_sb[:, c0:c1], agin_sb[:, c0:c1], w1s[:]
                    )
                nc.scalar.dma_start(agin[:, c0:c1], agin_sb[:, c0:c1])

            residents_pre = list(range(resg))
            streams_pre = list(range(resg, len(grps)))
            if l2order == "streamfirst":
                _order_preview = streams_pre[:abufs] + residents_pre + streams_pre[abufs:]
            elif l2order == "weave":
                _order_preview = []
                for i in range(2):
                    if i < len(streams_pre):
                        _order_preview.append(streams_pre[i])
                    if i < len(residents_pre):
                        _order_preview.append(residents_pre[i])
                _order_preview += residents_pre[2:] + streams_pre[2:]
            else:
                _order_preview = None

            if nocc:
                nc.scalar.dma_start(agout[0:128, :], agin[:])
            else:
                nc.gpsimd.collective_compute(
                    "AllGather",
                    mybir.AluOpType.bypass,
                    replica_groups=[list(range(NCORES))],
                    ins=[agin.opt()],
                    outs=[agout.opt()],
                )
            # agout rank blocks laid side by side in the free dim are exactly
            # layer-2's lhsT tiles in the same padded rank-block order A uses.
            rank_order = []
            for gi in _order_preview:
                k0, k1 = grps[gi]
                for r in ((k0 * 128) // SP, ((k1 * 128) - 1) // SP):
                    if r not in rank_order:
                        rank_order.append(r)
            for r in range(NCORES):
                if r not in rank_order:
                    rank_order.append(r)
            for r in rank_order:
                nc.sync.dma_start(
                    x1s[:, r * SP:(r + 1) * SP],
                    agout[r * 128:(r + 1) * 128, :],
                )

            # ---- layer 2 (X-stationary; psum is [feat, dst]) ----
            # Interleave: residents first (rank-0 x1 chunk arrives first),
            # streamed groups spread out so their DMAs pipeline through the
            # abufs slots while PE chews residents.
            psum2 = ps.tile([128, SP], f32, tag="acc2")
            residents = residents_pre
            streams = streams_pre
            if l2order == "streamfirst":
                order = streams[:abufs] + residents + streams[abufs:]
            elif l2order == "weave":
                # s0 r0 s1 r1 r2 ... then remaining streams at the tail
                order = []
                for i in range(2):
                    if i < len(streams):
                        order.append(streams[i])
                    if i < len(residents):
                        order.append(residents[i])
                order += residents[2:] + streams[2:]
            else:
                order = []
                ri, si = 0, 0
                pattern = [0, 0, 1, 0, 1, 0, 1, 0, 1, 0, 1, 0]  # 1 = stream
                for p in pattern[:len(grps)]:
                    if p and si < len(streams):
                        order.append(streams[si]); si += 1
                    elif ri < len(residents):
                        order.append(residents[ri]); ri += 1
                order += residents[ri:] + streams[si:]
                if order[-1] in streams:
                    for i in range(len(order) - 2, -1, -1):
                        if order[i] in residents:
                            order.append(order.pop(i))
                            break

            ob = ev.tile([128, SP], f32, tag="ob")
            first = True
            for oi, gi in enumerate(order):
                k0, k1 = grps[gi]
                ab = res_tiles[gi] if gi < resg else fetch_group(gi)
                last_grp = oi == len(order) - 1
                if not last_grp:
                    for k in range(k0, k1):
                        kk = k - k0
                        lhsT = x1s[:, k * 128:(k + 1) * 128]
                        for c0, cn in ((0, 512), (512, 512), (1024, 256)):
                            nc.tensor.matmul(
                                psum2[:, c0:c0 + cn],
                                lhsT,
                                ab[:, kk * SP + c0: kk * SP + c0 + cn],
                                start=first, stop=False,
                            )
                        first = False
                else:
                    # final group: bank-outer so each psum2 bank completes
                    # (stop=True) early and its eviction overlaps the rest
                    for c0, cn in ((0, 512), (512, 512), (1024, 256)):
                        for k in range(k0, k1):
                            kk = k - k0
                            nc.tensor.matmul(
                                psum2[:, c0:c0 + cn],
                                x1s[:, k * 128:(k + 1) * 128],
                                ab[:, kk * SP + c0: kk * SP + c0 + cn],
                                start=False, stop=(k == k1 - 1),
                            )
                        nc.vector.tensor_copy(
                            ob[:, c0:c0 + cn], psum2[:, c0:c0 + cn]
                        )
                        nc.sync.dma_start(
                            out[:, c0:c0 + cn], ob[:, c0:c0 + cn]
                        )

    nc.compile()
    return nc


def get_program(nocc=False, skip=(), u8=True, resg=RESG, abufs=2,
                w1_ones=True, l2order="streamfirst", gsizes=GSIZES):
    key = ("nc", nocc, tuple(skip), u8, resg, abufs, w1_ones, l2order,
           tuple(gsizes))
    if key not in _PROG_CACHE:
        _PROG_CACHE[key] = _build_program(nocc, skip, u8, resg, abufs,
                                          w1_ones, l2order, gsizes)
    return _PROG_CACHE[key]


def _node_perm():
    """Padded rank-block src ordering: slot i <-> (rank r = i//1280,
    local q = i%1280); global node r*1250+q for q<1250, else pad."""
    i2 = np.arange(NPAD)
    r2 = i2 // SP
    loc = i2 % SP
    node = r2 * S + loc
    valid = loc < S
    return np.where(valid, node, 0), valid


def build_in_maps(x, src, dst, vals, W, u8=True):
    """Host-side prep: dense A^T shard (u8 per-column quantized) + x0."""
    import scipy.sparse as sp

    x = np.asarray(x, np.float32)
    src = np.asarray(src, np.int64)
    dst = np.asarray(dst, np.int64)
    vals = np.asarray(vals, np.float32)
    W = np.asarray(W, np.float32)

    # A[dst, src] = sum of vals  ->  we build AT[src, dst]
    AT = sp.coo_matrix((vals, (src, dst)), shape=(N, N)).toarray()

    node2, valid2 = _node_perm()

    xw = x * W[0][None, :]
    x0p = np.zeros((NPAD, D), np.float32)
    x0p[valid2] = xw[node2[valid2]]
    x0h = np.ascontiguousarray(
        x0p.reshape(KT, 128, D).transpose(1, 0, 2).reshape(128, KT * D)
    ).astype(np.float16)

    w1brow = np.ascontiguousarray(
        np.tile(W[1][None, :], (128, 1))
    ).astype(np.float16)

    in_maps = []
    steps = []
    for c in range(NCORES):
        ATc = AT[:, c * S:(c + 1) * S]  # [N, S] float32
        colmax = np.maximum(ATc.max(axis=0), 1e-9)
        step = colmax / 255.0
        if u8:
            Aq = np.clip(np.rint(ATc * (1.0 / step)[None, :]), 0, 255).astype(
                np.uint8
            )
        else:
            Aq = (ATc * (1.0 / step)[None, :]).astype(np.float16)
        Ap = np.zeros((NPAD, SP), Aq.dtype)
        Ap[valid2, :S] = Aq[node2[valid2]]
        step_pad = np.zeros(SP, np.float32)
        step_pad[:S] = step
        steps.append(step_pad)
        # csc[p, t] = dequant scale of dst slot t*128+p
        csc_tile = np.ascontiguousarray(step_pad.reshape(10, 128).T).astype(
            np.float32
        )
        in_maps.append(
            {
                "a": np.ascontiguousarray(Ap.reshape(KT, 128, SP)),
                "x0": x0h,
                "csc": csc_tile,
                "w1b": w1brow,
            }
        )
    return in_maps, steps


def assemble_output(results, steps):
    outs = []
    for c in range(NCORES):
        ot = np.asarray(results[c]["out"], np.float32)  # [128, SP] feat-major
        ot = ot * steps[c][None, :]  # per-dst dequant (layer-2)
        outs.append(ot[:, :S].T)
    return np.ascontiguousarray(np.concatenate(outs, axis=0))


def kernel(x, src, dst, vals, W):
    from concourse import bass_utils

    w1_ones = bool(np.all(np.asarray(W)[1] == 1.0))
    nc = get_program(w1_ones=w1_ones)
    in_maps, steps = build_in_maps(x, src, dst, vals, W)
    # The axon terminal can wedge when a different program was loaded
    # earlier in its lifetime; a retry lands on the restarted terminal.
    last_err = None
    for _attempt in range(3):
        try:
            res = bass_utils.run_bass_kernel_spmd(
                nc, in_maps, core_ids=list(range(NCORES))
            )
            return assemble_output(res.results, steps)
        except Exception as e:  # noqa: BLE001
            last_err = e
            import time as _time

            _time.sleep(10.0)
    raise last_err
